# revision 12
# baseline (speedup 1.0000x reference)
"""GumbelQuantizer (VQ codebook, K=8192, D=256, N=8192 tokens) on 8 trn2 cores.

Algorithm (exact forward-pass reduction of the reference):
  scores[n,k] = gumbel[n,k] - ||w_k||^2 + 2*flat_n.w_k   (drop per-token ||flat||^2:
                constant across k, cannot change the argmax)
  idx[n]      = argmax_k scores[n,k]
  everything else (z_q gather, STE output, vq_loss, perplexity) follows
  exactly from idx + the original inputs and is done on host in numpy.

Device work per core (data-parallel over tokens, 1024 tokens/core):
  - matmul (2*flatT in fp16).T @ (wT * 8192 in fp16) -> PSUM fp32.  The w
    prescale keeps the tiny codebook values in fp16 normal range; the
    eviction copy applies the exact 2^-13 downscale.  Matmul-term abs error
    ~1.5e-6 vs gumbel's O(1) top-2 gaps -> argmax flips are ~1e-6 probable.
  - ScalarE evicts PSUM->SBUF scores in 2048-col strips (4 psum banks),
    applying scale=1/8192
  - SWDGE DMA with accum_op=add streams gumbel' (= gumbel - ||w||^2, host
    prep) from DRAM, adding elementwise onto the scores in fp32 (CCE add)
  - VectorE max8 + find_index8 per half-tile -> top value + argmax; halves
    are combined on host (exact, ties resolve to the lower index)
"""

import sys

for _p in ("/opt/trn_rl_repo",):
    if _p not in sys.path:
        sys.path.insert(0, _p)

import numpy as np

from concourse import bacc, bass, mybir
from concourse import tile
from concourse.bass_utils import run_bass_kernel_spmd

K = 8192
D = 256
B, H, W = 8, 32, 32
N = B * H * W          # 8192 tokens
NCORES = 8
NLOC = N // NCORES     # 1024 tokens per core
NTILES = NLOC // 128   # 8 token tiles of 128
KCHUNK = 512           # matmul moving-operand chunk (one psum bank out)
EVCHUNK = 2048          # psum eviction strip (4 banks)
KHALF = K // 2         # max8/find_index8 granularity
GCHUNK = 2048          # CCE accumulate descriptor limit
WSCALE = 8192.0        # exact power of two

_PROGRAM = None


def _build_program():
    nc = bacc.Bacc(None, target_bir_lowering=False)
    f32 = mybir.dt.float32
    f16 = mybir.dt.float16
    x2t_in = nc.declare_dram_parameter("x2t", [D, NLOC], f16, isOutput=False)
    wt_in = nc.declare_dram_parameter("wt", [D, K], f16, isOutput=False)
    gp_in = nc.declare_dram_parameter("gp", [NLOC, K], f32, isOutput=False)
    # per tile, per half: top-8 values + top-8 indices
    val_out = nc.declare_dram_parameter(
        "val8", [NTILES, 2, 128, 8], f32, isOutput=True
    )
    idx_out = nc.declare_dram_parameter(
        "idx8", [NTILES, 2, 128, 8], mybir.dt.uint32, isOutput=True
    )

    with tile.TileContext(nc) as tc:
        with (
            tc.tile_pool(name="const", bufs=1) as const,
            tc.tile_pool(name="scores", bufs=3) as scores_pool,
            tc.tile_pool(name="psum", bufs=2, space="PSUM") as psum_pool,
            tc.tile_pool(name="outs", bufs=8) as outs,
        ):
            xa = const.tile([128, NLOC], f16)
            xb = const.tile([128, NLOC], f16)
            wa = const.tile([128, K], f16)
            wb = const.tile([128, K], f16)
            nc.sync.dma_start(out=xa, in_=x2t_in[0:128, :])
            nc.sync.dma_start(out=xb, in_=x2t_in[128:256, :])
            # chunked weight loads so the first matmuls start early
            for c0 in range(0, K, EVCHUNK):
                nc.sync.dma_start(
                    out=wa[:, c0 : c0 + EVCHUNK], in_=wt_in[0:128, c0 : c0 + EVCHUNK]
                )
                nc.sync.dma_start(
                    out=wb[:, c0 : c0 + EVCHUNK], in_=wt_in[128:256, c0 : c0 + EVCHUNK]
                )

            for t in range(NTILES):
                sc = scores_pool.tile([128, K], f32, tag="sc")
                tok = slice(t * 128, (t + 1) * 128)
                for e0 in range(0, K, EVCHUNK):
                    ps = psum_pool.tile([128, EVCHUNK], f32, tag="ps")
                    for j0 in range(0, EVCHUNK, KCHUNK):
                        ks = slice(e0 + j0, e0 + j0 + KCHUNK)
                        pslice = ps[:, j0 : j0 + KCHUNK]
                        nc.tensor.matmul(
                            pslice, xa[:, tok], wa[:, ks], start=True, stop=False
                        )
                        nc.tensor.matmul(
                            pslice, xb[:, tok], wb[:, ks], start=False, stop=True
                        )
                    # evict 4 banks at once, applying the exact 2^-13 downscale
                    nc.scalar.activation(
                        sc[:, e0 : e0 + EVCHUNK],
                        ps,
                        mybir.ActivationFunctionType.Copy,
                        scale=1.0 / WSCALE,
                    )
                    # immediately stream this strip's gumbel' on top (CCE add)
                    nc.gpsimd.dma_start(
                        out=sc[:, e0 : e0 + EVCHUNK],
                        in_=gp_in[tok, e0 : e0 + EVCHUNK],
                        accum_op=mybir.AluOpType.add,
                    )
                for half in range(2):
                    h0 = half * KHALF
                    v8 = outs.tile([128, 8], f32, tag="v8")
                    i8 = outs.tile([128, 8], mybir.dt.uint32, tag="i8")
                    nc.vector.max(v8, sc[:, h0 : h0 + KHALF])
                    nc.vector.max_index(i8, v8, sc[:, h0 : h0 + KHALF])
                    nc.sync.dma_start(out=val_out[t, half], in_=v8)
                    nc.sync.dma_start(out=idx_out[t, half], in_=i8)
    nc.compile()
    return nc


def _get_program():
    global _PROGRAM
    if _PROGRAM is None:
        _PROGRAM = _build_program()
    return _PROGRAM


def _prep_inputs(z_e, weight, gumbel):
    """Host prep: per-core input maps."""
    w2 = (weight.astype(np.float64) ** 2).sum(axis=1).astype(np.float32)  # [K]
    wt16 = np.ascontiguousarray((weight.T * WSCALE).astype(np.float16))  # [D, K]
    in_maps = []
    for c in range(NCORES):
        x2 = (2.0 * z_e[c]).reshape(D, NLOC)  # exact: *2
        gp = gumbel[c * NLOC : (c + 1) * NLOC, :] - w2[None, :]
        in_maps.append(
            {
                "x2t": np.ascontiguousarray(x2.astype(np.float16)),
                "wt": wt16,
                "gp": np.ascontiguousarray(gp, dtype=np.float32),
            }
        )
    return in_maps


def _postprocess(idx_all, z_e, weight):
    """Exact host reconstruction of all four outputs from the argmax indices."""
    z_q_flat = weight[idx_all]  # [N, D] — bit-exact vs one-hot matmul
    z_e_p = np.ascontiguousarray(z_e.transpose(0, 2, 3, 1))  # [B,H,W,D] f32
    z_q = z_q_flat.reshape(B, H, W, D)
    diff = z_q - z_e_p  # fp32 elementwise, mirrors jnp rounding
    z_q_ste = np.ascontiguousarray((z_e_p + diff).transpose(0, 3, 1, 2))
    vq_loss = np.float32(1.25 * np.mean(diff.astype(np.float64) ** 2))
    counts = np.bincount(idx_all, minlength=K).astype(np.float64)
    avg_probs = counts / float(N)
    perplexity = np.float32(np.exp(-np.sum(avg_probs * np.log(avg_probs + 1e-10))))
    encoding_indices = idx_all.astype(np.int32)
    return z_q_ste, vq_loss, perplexity, encoding_indices


def kernel(z_e, weight, gumbel, _trace=False, _trace_kwargs=None):
    nc = _get_program()
    in_maps = _prep_inputs(z_e, weight, gumbel)
    res = run_bass_kernel_spmd(
        nc, in_maps, core_ids=list(range(NCORES)), trace=_trace,
        **(_trace_kwargs or {}),
    )
    idx_parts = []
    for c in range(NCORES):
        v8 = res.results[c]["val8"].astype(np.float32)  # [NTILES, 2, 128, 8]
        i8 = res.results[c]["idx8"]                     # [NTILES, 2, 128, 8]
        va, vb = v8[:, 0, :, 0], v8[:, 1, :, 0]         # [NTILES, 128]
        ia, ib = i8[:, 0, :, 0].astype(np.int64), i8[:, 1, :, 0].astype(np.int64)
        # first-occurrence tie-break: half A wins ties
        idx = np.where(va >= vb, ia, ib + KHALF)        # [NTILES, 128]
        idx_parts.append(idx.reshape(-1))
    idx_all = np.concatenate(idx_parts)  # [N]
    out = _postprocess(idx_all, z_e, weight)
    if _trace:
        return out, res
    return out


# revision 15
# speedup vs baseline: 1.1834x; 1.1834x over previous
"""GumbelQuantizer (VQ codebook, K=8192, D=256, N=8192 tokens) on 8 trn2 cores.

Algorithm (exact forward-pass reduction of the reference):
  scores[n,k] = gumbel[n,k] - ||w_k||^2 + 2*flat_n.w_k   (drop per-token ||flat||^2:
                constant across k, cannot change the argmax)
  idx[n]      = argmax_k scores[n,k]
  everything else (z_q gather, STE output, vq_loss, perplexity) follows
  exactly from idx + the original inputs and is done on host in numpy.

Device work per core (data-parallel over tokens, 1024 tokens/core):
  - matmul (2*flatT in fp16).T @ (wT * 8192 in fp16) -> PSUM fp32.  The w
    prescale keeps the tiny codebook values in fp16 normal range; the
    eviction copy applies the exact 2^-13 downscale.  Matmul-term abs error
    ~1.5e-6 vs gumbel's O(1) top-2 gaps -> argmax flips are ~1e-6 probable.
  - ScalarE evicts PSUM->SBUF scores in 2048-col strips (4 psum banks),
    applying scale=1/8192
  - SWDGE DMA with accum_op=add streams gumbel' (= gumbel - ||w||^2, host
    prep) from DRAM, adding elementwise onto the scores in fp32 (CCE add)
  - VectorE max8 + find_index8 per half-tile -> top value + argmax; halves
    are combined on host (exact, ties resolve to the lower index)
"""

import sys

for _p in ("/opt/trn_rl_repo",):
    if _p not in sys.path:
        sys.path.insert(0, _p)

import numpy as np

from concourse import bacc, bass, mybir
from concourse import tile
from concourse.bass_utils import run_bass_kernel_spmd

K = 8192
D = 256
B, H, W = 8, 32, 32
N = B * H * W          # 8192 tokens
NCORES = 8
NLOC = N // NCORES     # 1024 tokens per core
NTILES = NLOC // 128   # 8 token tiles of 128
KCHUNK = 512           # matmul moving-operand chunk (one psum bank out)
EVCHUNK = 2048          # psum eviction strip (4 banks)
KHALF = K // 2         # max8/find_index8 granularity
GCHUNK = 2048          # CCE accumulate descriptor limit
WSCALE = 8192.0        # exact power of two

_PROGRAM = None


def _build_program():
    nc = bacc.Bacc(None, target_bir_lowering=False)
    f32 = mybir.dt.float32
    f16 = mybir.dt.float16
    x2t_in = nc.declare_dram_parameter("x2t", [D, NLOC], f16, isOutput=False)
    wt_in = nc.declare_dram_parameter("wt", [D, K], f16, isOutput=False)
    ghi_in = nc.declare_dram_parameter("ghi", [NLOC, K], f16, isOutput=False)
    glo_in = nc.declare_dram_parameter("glo", [NLOC, K], f16, isOutput=False)
    id_in = nc.declare_dram_parameter("ident", [128, 128], f16, isOutput=False)
    # per tile, per half: top-8 values + top-8 indices
    val_out = nc.declare_dram_parameter(
        "val8", [NTILES, 2, 128, 8], f32, isOutput=True
    )
    idx_out = nc.declare_dram_parameter(
        "idx8", [NTILES, 2, 128, 8], mybir.dt.uint32, isOutput=True
    )

    with tile.TileContext(nc) as tc:
        with (
            tc.tile_pool(name="const", bufs=1) as const,
            tc.tile_pool(name="scores", bufs=3) as scores_pool,
            tc.tile_pool(name="gst", bufs=6) as gst_pool,
            tc.tile_pool(name="psum", bufs=2, space="PSUM") as psum_pool,
            tc.tile_pool(name="outs", bufs=8) as outs,
        ):
            xa = const.tile([128, NLOC], f16)
            xb = const.tile([128, NLOC], f16)
            ident = const.tile([128, 128], f16)  # 8192 * I
            wa = const.tile([128, K], f16)
            wb = const.tile([128, K], f16)
            nc.sync.dma_start(out=xa, in_=x2t_in[0:128, :])
            nc.sync.dma_start(out=xb, in_=x2t_in[128:256, :])
            nc.sync.dma_start(out=ident, in_=id_in[:, :])
            # chunked weight loads so the first matmuls start early
            for c0 in range(0, K, EVCHUNK):
                nc.sync.dma_start(
                    out=wa[:, c0 : c0 + EVCHUNK], in_=wt_in[0:128, c0 : c0 + EVCHUNK]
                )
                nc.sync.dma_start(
                    out=wb[:, c0 : c0 + EVCHUNK], in_=wt_in[128:256, c0 : c0 + EVCHUNK]
                )

            for t in range(NTILES):
                sc = scores_pool.tile([128, K], f32, tag="sc")
                tok = slice(t * 128, (t + 1) * 128)
                for e0 in range(0, K, EVCHUNK):
                    # prefetch this strip's gumbel hi/lo (plain HWDGE, f16)
                    ghi_t = gst_pool.tile([128, EVCHUNK], f16, tag="ghi")
                    glo_t = gst_pool.tile([128, EVCHUNK], f16, tag="glo")
                    nc.sync.dma_start(out=ghi_t, in_=ghi_in[tok, e0 : e0 + EVCHUNK])
                    nc.sync.dma_start(out=glo_t, in_=glo_in[tok, e0 : e0 + EVCHUNK])
                    ps = psum_pool.tile([128, EVCHUNK], f32, tag="ps")
                    for j0 in range(0, EVCHUNK, KCHUNK):
                        ks = slice(e0 + j0, e0 + j0 + KCHUNK)
                        js = slice(j0, j0 + KCHUNK)
                        pslice = ps[:, js]
                        nc.tensor.matmul(
                            pslice, xa[:, tok], wa[:, ks], start=True, stop=False
                        )
                        nc.tensor.matmul(
                            pslice, xb[:, tok], wb[:, ks], start=False, stop=False
                        )
                        # gumbel' enters via scaled-identity matmuls: psum
                        # accumulates 8192*(ghi+glo), matching the w prescale
                        nc.tensor.matmul(
                            pslice, ident, ghi_t[:, js], start=False, stop=False
                        )
                        nc.tensor.matmul(
                            pslice, ident, glo_t[:, js], start=False, stop=True
                        )
                    # evict 4 banks at once, applying the exact 2^-13 downscale
                    nc.scalar.activation(
                        sc[:, e0 : e0 + EVCHUNK],
                        ps,
                        mybir.ActivationFunctionType.Copy,
                        scale=1.0 / WSCALE,
                    )
                for half in range(2):
                    h0 = half * KHALF
                    v8 = outs.tile([128, 8], f32, tag="v8")
                    i8 = outs.tile([128, 8], mybir.dt.uint32, tag="i8")
                    nc.vector.max(v8, sc[:, h0 : h0 + KHALF])
                    nc.vector.max_index(i8, v8, sc[:, h0 : h0 + KHALF])
                    nc.sync.dma_start(out=val_out[t, half], in_=v8)
                    nc.sync.dma_start(out=idx_out[t, half], in_=i8)
    nc.compile()
    return nc


def _get_program():
    global _PROGRAM
    if _PROGRAM is None:
        _PROGRAM = _build_program()
    return _PROGRAM


def _prep_inputs(z_e, weight, gumbel):
    """Host prep: per-core input maps."""
    w2 = (weight.astype(np.float64) ** 2).sum(axis=1).astype(np.float32)  # [K]
    wt16 = np.ascontiguousarray((weight.T * WSCALE).astype(np.float16))  # [D, K]
    ident = (WSCALE * np.eye(128, dtype=np.float32)).astype(np.float16)
    in_maps = []
    for c in range(NCORES):
        x2 = (2.0 * z_e[c]).reshape(D, NLOC)  # exact: *2
        gp = gumbel[c * NLOC : (c + 1) * NLOC, :] - w2[None, :]
        ghi = gp.astype(np.float16)
        glo = (gp - ghi.astype(np.float32)).astype(np.float16)
        in_maps.append(
            {
                "x2t": np.ascontiguousarray(x2.astype(np.float16)),
                "wt": wt16,
                "ghi": np.ascontiguousarray(ghi),
                "glo": np.ascontiguousarray(glo),
                "ident": ident,
            }
        )
    return in_maps


def _postprocess(idx_all, z_e, weight):
    """Exact host reconstruction of all four outputs from the argmax indices."""
    z_q_flat = weight[idx_all]  # [N, D] — bit-exact vs one-hot matmul
    z_e_p = np.ascontiguousarray(z_e.transpose(0, 2, 3, 1))  # [B,H,W,D] f32
    z_q = z_q_flat.reshape(B, H, W, D)
    diff = z_q - z_e_p  # fp32 elementwise, mirrors jnp rounding
    z_q_ste = np.ascontiguousarray((z_e_p + diff).transpose(0, 3, 1, 2))
    vq_loss = np.float32(1.25 * np.mean(diff.astype(np.float64) ** 2))
    counts = np.bincount(idx_all, minlength=K).astype(np.float64)
    avg_probs = counts / float(N)
    perplexity = np.float32(np.exp(-np.sum(avg_probs * np.log(avg_probs + 1e-10))))
    encoding_indices = idx_all.astype(np.int32)
    return z_q_ste, vq_loss, perplexity, encoding_indices


def kernel(z_e, weight, gumbel, _trace=False, _trace_kwargs=None):
    nc = _get_program()
    in_maps = _prep_inputs(z_e, weight, gumbel)
    res = run_bass_kernel_spmd(
        nc, in_maps, core_ids=list(range(NCORES)), trace=_trace,
        **(_trace_kwargs or {}),
    )
    idx_parts = []
    for c in range(NCORES):
        v8 = res.results[c]["val8"].astype(np.float32)  # [NTILES, 2, 128, 8]
        i8 = res.results[c]["idx8"]                     # [NTILES, 2, 128, 8]
        va, vb = v8[:, 0, :, 0], v8[:, 1, :, 0]         # [NTILES, 128]
        ia, ib = i8[:, 0, :, 0].astype(np.int64), i8[:, 1, :, 0].astype(np.int64)
        # first-occurrence tie-break: half A wins ties
        idx = np.where(va >= vb, ia, ib + KHALF)        # [NTILES, 128]
        idx_parts.append(idx.reshape(-1))
    idx_all = np.concatenate(idx_parts)  # [N]
    out = _postprocess(idx_all, z_e, weight)
    if _trace:
        return out, res
    return out
